# revision 14
# baseline (speedup 1.0000x reference)
"""Causal multi-head attention kernel for 8 trn2 NeuronCores.

Problem: x[2,2048,1024], 16 heads of dim 64, causal softmax(q k^T / sqrt(1024)) v,
then output projection. Sharding: data-parallel over batch (4 cores per batch),
tensor-parallel over heads (4 heads per core). Each core produces a partial
output (its heads' contribution through Wout); the host sums the 4 partials per
batch and adds b_out.

Datapath is bf16 (inputs cast on host) with fp32 PSUM accumulation; the
rel-err budget is 2e-2 and bf16 keeps the end-to-end error ~5e-3. bf16 buys:
full PE rate at any free size (no >=256 fp32r padding), DMA-engine XBAR
transpose of x (the whole PE transpose phase is gone), half the DMA bytes,
and 2x DVE modes on the mask multiplies.

Per-core device program (SPMD, per-core data arrives via input tensors):
  1. xT [d on partitions, n free] arrives directly via dma_start_transpose
     (16 half-slab XBAR transposes), overlapped with weight DMAs.
  2. Projections: qT/kT [dh on partitions, n free] (2-head groups of 128
     partitions), v natural [n on partitions] augmented with a ones column so
     the attention matmul also produces softmax row-sums.
  3. Per head, per 1024-wide i-chunk: S^T[j-block, i] = kT_j^T . qT_i on PE,
     exp((1/32) S) on ACT straight out of PSUM (logits are bounded, no
     max-subtraction needed) writing bf16, triangular mask multiply only on
     the diagonal 128x128 block, then O^T[dh+1, i] += V_aug^T . P^T
     accumulated over j-blocks in PSUM. Block-causality skips all j>i blocks,
     and segments are trimmed exactly to the causal offset.
  4. Normalize by the row-sums (reciprocal_approx_fast + broadcast multiply),
     output projection with two 2-head pairs packed to a full K=128
     contraction.
"""

import os

import numpy as np

B, N, D, H = 2, 2048, 1024, 16
DH = D // H  # 64
SCALE = float(D) ** -0.5
NCORES = 8
HPC = 4  # heads per core
IC = 1024  # i-chunk width in attention phase
NB = N // 128  # 16 j/n blocks
KT = D // 128  # 8 contraction tiles
# v columns per head: 64 data cols + 64 ones cols. The ones columns make the
# attention matmul write the softmax row-sum replicated on PSUM partitions
# 64..127, so normalization is a plain elementwise reciprocal+multiply
# (partitions 0..63 / partitions 64..127) with no partition-broadcast needed.
VW = 2 * DH  # 128

_cached = {}
_last_results = None


def _build_program():
    import concourse.bacc as bacc
    import concourse.mybir as mybir
    import concourse.tile as tile

    f32 = mybir.dt.float32
    bf16 = mybir.dt.bfloat16
    EXP = mybir.ActivationFunctionType.Exp

    nc = bacc.Bacc()

    xb = nc.dram_tensor("xb", [N, D], bf16, kind="ExternalInput")
    wq = nc.dram_tensor("wq", [D, HPC * DH], bf16, kind="ExternalInput")
    wk = nc.dram_tensor("wk", [D, HPC * DH], bf16, kind="ExternalInput")
    wv = nc.dram_tensor("wv", [D, HPC * DH], bf16, kind="ExternalInput")
    wo = nc.dram_tensor("wo", [HPC * DH, D], bf16, kind="ExternalInput")
    tri = nc.dram_tensor("tri", [128, 128], bf16, kind="ExternalInput")
    outp = nc.dram_tensor("outp", [N, D], bf16, kind="ExternalOutput")

    with tile.TileContext(nc) as tc:
        with (
            tc.tile_pool(name="const", bufs=1) as const_pool,
            tc.tile_pool(name="wts", bufs=1) as wts_pool,
            tc.tile_pool(name="big", bufs=1) as big_pool,
        ):
            # Input DMA is the kernel lead-in and every ns of it delays the
            # PE's first matmul, so it is split across both HWDGE queues
            # (SP="sync" and ACT="scalar") in consumption order: wv first
            # (the V projection runs first), then the x^T XBAR-transpose
            # half-slabs interleaved with wq/wk, and the attention-only
            # constants (tri, wo) last.
            tri_sb = const_pool.tile([128, 128], bf16, name="tri_sb", tag="tri_sb")
            wo_sb = []
            for g in range(2):
                t = const_pool.tile([128, D], bf16, name=f"wo{g}", tag=f"wo{g}")
                wo_sb.append(t)
            wq_sb, wk_sb, wv_sb = [], [], []
            for nm, lst in (("wq", wq_sb), ("wk", wk_sb), ("wv", wv_sb)):
                for r in range(KT):
                    t = wts_pool.tile(
                        [128, HPC * DH], bf16, name=f"{nm}{r}", tag=f"{nm}{r}"
                    )
                    lst.append(t)
            xT = []
            for r in range(KT):
                t = big_pool.tile([128, N], bf16, name=f"xT{r}", tag=f"xT{r}")
                xT.append(t)

            # All XBAR transposes go on ONE queue (concurrent transposes on
            # both HWDGE queues corrupt each other); the weight loads stream
            # in parallel on the other queue.
            for half in range(2):
                hsl = slice(1024 * half, 1024 * (half + 1))
                for r in range(KT):
                    nc.sync.dma_start_transpose(
                        out=xT[r][:, hsl], in_=xb[hsl, 128 * r : 128 * (r + 1)]
                    )
            for r in range(KT):
                nc.scalar.dma_start(out=wv_sb[r], in_=wv[128 * r : 128 * (r + 1), :])
            for r in range(KT):
                nc.scalar.dma_start(out=wq_sb[r], in_=wq[128 * r : 128 * (r + 1), :])
            for r in range(KT):
                nc.scalar.dma_start(out=wk_sb[r], in_=wk[128 * r : 128 * (r + 1), :])
            nc.scalar.dma_start(out=tri_sb, in_=tri[:, :])
            for g in range(2):
                nc.scalar.dma_start(
                    out=wo_sb[g], in_=wo[128 * g : 128 * (g + 1), :]
                )

            with (
                tc.tile_pool(name="pj", bufs=2, space="PSUM") as pj_pool,
                tc.tile_pool(name="osb", bufs=3) as osb_pool,
            ):
                qT, kT_ = [], []
                for g in range(2):
                    tq = big_pool.tile([128, N], bf16, name=f"qT{g}", tag=f"qT{g}")
                    tk = big_pool.tile([128, N], bf16, name=f"kT{g}", tag=f"kT{g}")
                    qT.append(tq)
                    kT_.append(tk)
                v_all = big_pool.tile(
                    [128, NB * HPC * VW], bf16, name="v_all", tag="v_all"
                )
                # ones columns for the row-sum trick: fill the whole tile with
                # 1.0; the projection copies below overwrite the data columns
                nc.vector.memset(v_all, 1.0)


                def _copy(eng, out, in_):
                    if eng == "scalar":
                        nc.scalar.copy(out=out, in_=in_)
                    else:
                        getattr(nc, eng).tensor_copy(out=out, in_=in_)

                def vproj_stream(nbs):
                    for nb in nbs:
                        pv = pj_pool.tile([128, HPC * DH], f32, name="pv", tag="pj")
                        for r in range(KT):
                            nc.tensor.matmul(
                                pv,
                                lhsT=xT[r][:, 128 * nb : 128 * (nb + 1)],
                                rhs=wv_sb[r],
                                start=(r == 0),
                                stop=(r == KT - 1),
                            )
                        base = nb * HPC * VW
                        for h in range(HPC):
                            _copy(
                                ("vector", "scalar", "scalar", "vector")[h],
                                v_all[:, base + VW * h : base + VW * h + DH],
                                pv[:, DH * h : DH * (h + 1)],
                            )
                        yield

                def qkproj_stream(g, copy_engines=("any",)):
                    ci = 0
                    for s4 in range(4):
                        sl = slice(512 * s4, 512 * (s4 + 1))
                        for w_sb, dst in ((wq_sb, qT[g]), (wk_sb, kT_[g])):
                            pq = pj_pool.tile([128, 512], f32, name="pq", tag="pj")
                            for r in range(KT):
                                nc.tensor.matmul(
                                    pq,
                                    lhsT=w_sb[r][:, 128 * g : 128 * (g + 1)],
                                    rhs=xT[r][:, sl],
                                    start=(r == 0),
                                    stop=(r == KT - 1),
                                )
                            _copy(copy_engines[ci % len(copy_engines)], dst[:, sl], pq)
                            ci += 1
                            yield

                # ---------------- phase B: projections ----------------
                # emission order tracks the DMA half-slab arrival order; only
                # what attention head 0 cp0 needs (V nb0..7 + all of qk g0)
                # runs here — the rest of V and all of g1 fill PE slack
                # inside the attention loop.
                for _ in vproj_stream(range(8)):
                    pass
                for _ in qkproj_stream(0, copy_engines=("vector", "scalar")):
                    pass

                # ---------------- phase C: attention with interleaved
                # ---------------- g1 projections and output projection ------
                OT = []
                for g in range(2):
                    t = big_pool.tile([128, N], bf16, name=f"OT{g}", tag=f"OT{g}")
                    OT.append(t)

                def outproj_stream(nbs, copy_engines):
                    ci = 0
                    for nb in nbs:
                        nsl = slice(128 * nb, 128 * (nb + 1))
                        for s in range(2):
                            po = pj_pool.tile([128, 512], f32, name="po", tag="pj")
                            for g in range(2):
                                nc.tensor.matmul(
                                    po,
                                    lhsT=OT[g][:, nsl],
                                    rhs=wo_sb[g][:, 512 * s : 512 * (s + 1)],
                                    start=(g == 0),
                                    stop=(g == 1),
                                )
                            ob = osb_pool.tile([128, 512], bf16, name="ob", tag="osb")
                            _copy(copy_engines[ci % len(copy_engines)], ob, po)
                            ci += 1
                            nc.sync.dma_start(
                                out=outp[nsl, 512 * s : 512 * (s + 1)], in_=ob
                            )
                            yield

                with (
                    tc.tile_pool(name="pS", bufs=2, space="PSUM") as pS_pool,
                    tc.tile_pool(name="pO", bufs=1, space="PSUM") as pO_pool,
                    tc.tile_pool(name="att", bufs=4) as att_pool,
                ):
                    # A@V emission lags the QK/exp emission by DELAY jb-steps
                    # so the in-order PE never stalls on the ACT exp; the
                    # PE-dense projection streams above are pulled in between
                    # attention steps to fill the remaining PE idle time.
                    DELAY = 2
                    pend = []

                    def drain(n):
                        while len(pend) > n:
                            pend.pop(0)()

                    def attention_stream():
                        for h in range(HPC):
                            g, row = h // 2, 64 * (h % 2)
                            for cp in range(2):
                                jd, jmax = 8 * cp, 8 * (cp + 1)
                                pO = pO_pool.tile(
                                    [128, IC], f32, name=f"pO{h}", tag="pO"
                                )
                                for jb in range(jmax):
                                    rel = jb - jd
                                    o = 128 * rel if rel > 0 else 0
                                    jsl = slice(128 * jb, 128 * (jb + 1))
                                    pS = pS_pool.tile(
                                        [128, IC], f32, name="pS", tag="pS"
                                    )
                                    pexp = att_pool.tile(
                                        [128, IC], bf16, name="pexp", tag="pexp"
                                    )
                                    # S^T = kT_j^T . qT_i in 512-wide segments
                                    # trimmed exactly to the causal offset
                                    for s in range(2):
                                        a = max(o, 512 * s)
                                        if a >= 512 * (s + 1):
                                            continue
                                        nc.tensor.matmul(
                                            pS[:, a : 512 * (s + 1)],
                                            lhsT=kT_[g][row : row + 64, jsl],
                                            rhs=qT[g][
                                                row : row + 64,
                                                IC * cp + a : IC * cp + 512 * (s + 1),
                                            ],
                                            start=True,
                                            stop=True,
                                        )
                                    nc.scalar.activation(
                                        out=pexp[:, o:IC],
                                        in_=pS[:, o:IC],
                                        func=EXP,
                                        scale=SCALE,
                                    )
                                    if rel >= 0:
                                        nc.vector.tensor_mul(
                                            pexp[:, o : o + 128],
                                            pexp[:, o : o + 128],
                                            tri_sb,
                                        )

                                    def av_unit(
                                        h=h, jb=jb, o=o, jd=jd, jmax=jmax,
                                        pO=pO, pexp=pexp,
                                    ):
                                        # seg1 first (never overlaps the
                                        # masked triangle); exact causal trim
                                        for s in (1, 0):
                                            hi = 512 * (s + 1)
                                            lo = max(o, 512 * s)
                                            if lo >= hi:
                                                continue
                                            vsl = slice(
                                                jb * HPC * VW + VW * h,
                                                jb * HPC * VW + VW * (h + 1),
                                            )
                                            nc.tensor.matmul(
                                                pO[:, lo:hi],
                                                lhsT=v_all[:, vsl],
                                                rhs=pexp[:, lo:hi],
                                                start=(jb == 0),
                                                stop=(
                                                    jb
                                                    == (jd + 3 if s == 0 else jmax - 1)
                                                ),
                                                skip_group_check=True,
                                            )

                                    pend.append(av_unit)
                                    drain(DELAY)
                                    yield (h, cp, jb)

                                # Normalize: split the O^T psum into two
                                # partition-0-aligned SBUF tiles (O rows on
                                # ACT, sum rows on DVE — reciprocal_approx_
                                # fast's custom ucode can't take a partition
                                # offset), one approx-reciprocal over the
                                # whole sums tile, then the broadcast
                                # multiplies in 256-col chunks spread through
                                # the pend queue.
                                pOcO = att_pool.tile(
                                    [64, IC], f32, name="pOcO", tag="pOcO", bufs=2
                                )
                                pOcS = att_pool.tile(
                                    [64, IC], f32, name="pOcS", tag="pOcS", bufs=2
                                )
                                rec = att_pool.tile(
                                    [64, IC], f32, name="rec", tag="rec", bufs=2
                                )

                                def copy_unit(pO=pO, pOcO=pOcO, pOcS=pOcS, rec=rec):
                                    nc.scalar.copy(out=pOcO, in_=pO[0:64, :])
                                    nc.vector.tensor_copy(
                                        out=pOcS, in_=pO[64:128, :]
                                    )
                                    nc.vector.reciprocal_approx_fast(
                                        out=rec, in_=pOcS
                                    )

                                pend.append(copy_unit)
                                for c4 in range(4):
                                    def norm_chunk(
                                        g=g, row=row, cp=cp, pOcO=pOcO,
                                        rec=rec, c4=c4,
                                    ):
                                        cs = slice(256 * c4, 256 * (c4 + 1))
                                        nc.vector.tensor_mul(
                                            OT[g][
                                                row : row + 64,
                                                IC * cp + 256 * c4 : IC * cp
                                                + 256 * (c4 + 1),
                                            ],
                                            pOcO[:, cs],
                                            rec[:, cs],
                                        )

                                    pend.append(norm_chunk)

                    att = attention_stream()
                    vfill = vproj_stream(range(8, 16))
                    g1fill = qkproj_stream(1, copy_engines=("vector",))
                    out0fill = outproj_stream(range(8), ("vector",))
                    v_done = False
                    g1_done = False
                    steps = 0
                    for h, cp, jb in att:
                        steps += 1
                        if not v_done:
                            v_done = next(vfill, "end") == "end"
                        elif not g1_done and steps % 3 == 0:
                            g1_done = next(g1fill, "end") == "end"
                        if h == 2 and not g1_done:
                            # g1 attention needs qT[1]/kT[1] complete
                            for _ in g1fill:
                                pass
                            g1_done = True
                        # nb 0..7 of the output projection read cp0 columns of
                        # OT; head 3's cp0 normalize leaves the pend queue at
                        # jb==DELAY of cp1, so only start pulling after that
                        if h == 3 and cp == 1 and jb > DELAY:
                            next(out0fill, None)
                            next(out0fill, None)
                    drain(0)
                    for _ in out0fill:
                        pass
                    for _ in outproj_stream(range(8, 16), ("scalar", "vector")):
                        pass

    nc.compile()
    return nc


def kernel(x, mask, Wq, Wkv, Wout, b_out):
    global _last_results
    import ml_dtypes
    from concourse.bass_utils import run_bass_kernel_spmd

    bf = ml_dtypes.bfloat16
    x = np.asarray(x, dtype=np.float32).astype(bf)
    Wq = np.asarray(Wq, dtype=np.float32).astype(bf)
    Wkv = np.asarray(Wkv, dtype=np.float32).astype(bf)
    Wout = np.asarray(Wout, dtype=np.float32).astype(bf)
    b_out = np.asarray(b_out, dtype=np.float32)

    if "nc" not in _cached:
        _cached["nc"] = _build_program()
    nc = _cached["nc"]

    jj, ii = np.mgrid[0:128, 0:128]
    tri = (jj <= ii).astype(bf)

    in_maps = []
    for c in range(NCORES):
        b = c // 4
        h0 = HPC * (c % 4)
        in_maps.append(
            {
                "xb": np.ascontiguousarray(x[b]),
                "wq": np.ascontiguousarray(Wq[:, DH * h0 : DH * (h0 + HPC)]),
                "wk": np.ascontiguousarray(Wkv[:, DH * h0 : DH * (h0 + HPC)]),
                "wv": np.ascontiguousarray(
                    Wkv[:, D + DH * h0 : D + DH * (h0 + HPC)]
                ),
                "wo": np.ascontiguousarray(Wout[DH * h0 : DH * (h0 + HPC), :]),
                "tri": tri,
            }
        )

    res = run_bass_kernel_spmd(
        nc,
        in_maps,
        core_ids=list(range(NCORES)),
        trace=bool(int(os.environ.get("KERNEL_TRACE", "0"))),
    )
    _last_results = res
    parts = [r["outp"] for r in res.results]
    out = np.empty((B, N, D), dtype=np.float32)
    for b in range(B):
        acc = np.asarray(parts[4 * b], dtype=np.float32)
        for c in range(4 * b + 1, 4 * b + 4):
            acc += np.asarray(parts[c], dtype=np.float32)
        out[b] = acc + b_out[None, :]
    return out


# revision 15
# speedup vs baseline: 1.2973x; 1.2973x over previous
"""Causal multi-head attention kernel for 8 trn2 NeuronCores.

Problem: x[2,2048,1024], 16 heads of dim 64, causal softmax(q k^T / sqrt(1024)) v,
then output projection. Sharding: data-parallel over batch (4 cores per batch),
tensor-parallel over heads (4 heads per core). Each core produces a partial
output (its heads' contribution through Wout); the host sums the 4 partials per
batch and adds b_out.

Datapath is bf16 (inputs cast on host) with fp32 PSUM accumulation; the
rel-err budget is 2e-2 and bf16 keeps the end-to-end error ~5e-3. bf16 buys:
full PE rate at any free size (no >=256 fp32r padding), DMA-engine XBAR
transpose of x (the whole PE transpose phase is gone), half the DMA bytes,
and 2x DVE modes on the mask multiplies.

Per-core device program (SPMD, per-core data arrives via input tensors):
  1. xT [d on partitions, n free] arrives directly via dma_start_transpose
     (16 half-slab XBAR transposes), overlapped with weight DMAs.
  2. Projections: qT/kT [dh on partitions, n free] (2-head groups of 128
     partitions), v natural [n on partitions] augmented with a ones column so
     the attention matmul also produces softmax row-sums.
  3. Per head, per 1024-wide i-chunk: S^T[j-block, i] = kT_j^T . qT_i on PE,
     exp((1/32) S) on ACT straight out of PSUM (logits are bounded, no
     max-subtraction needed) writing bf16, triangular mask multiply only on
     the diagonal 128x128 block, then O^T[dh+1, i] += V_aug^T . P^T
     accumulated over j-blocks in PSUM. Block-causality skips all j>i blocks,
     and segments are trimmed exactly to the causal offset.
  4. Normalize by the row-sums (reciprocal_approx_fast + broadcast multiply),
     output projection with two 2-head pairs packed to a full K=128
     contraction.
"""

import os

import numpy as np

B, N, D, H = 2, 2048, 1024, 16
DH = D // H  # 64
SCALE = float(D) ** -0.5
NCORES = 8
HPC = 4  # heads per core
IC = 1024  # i-chunk width in attention phase
NB = N // 128  # 16 j/n blocks
KT = D // 128  # 8 contraction tiles
# v columns per head: 64 data cols + 64 ones cols. The ones columns make the
# attention matmul write the softmax row-sum replicated on PSUM partitions
# 64..127, so normalization is a plain elementwise reciprocal+multiply
# (partitions 0..63 / partitions 64..127) with no partition-broadcast needed.
VW = 2 * DH  # 128

_cached = {}
_last_results = None


def _build_program():
    import concourse.bacc as bacc
    import concourse.mybir as mybir
    import concourse.tile as tile

    f32 = mybir.dt.float32
    bf16 = mybir.dt.bfloat16
    EXP = mybir.ActivationFunctionType.Exp

    nc = bacc.Bacc()

    xb = nc.dram_tensor("xb", [N, D], bf16, kind="ExternalInput")
    wq = nc.dram_tensor("wq", [D, HPC * DH], bf16, kind="ExternalInput")
    wk = nc.dram_tensor("wk", [D, HPC * DH], bf16, kind="ExternalInput")
    wv = nc.dram_tensor("wv", [D, HPC * DH], bf16, kind="ExternalInput")
    wo = nc.dram_tensor("wo", [HPC * DH, D], bf16, kind="ExternalInput")
    tri = nc.dram_tensor("tri", [128, 128], bf16, kind="ExternalInput")
    outp = nc.dram_tensor("outp", [N, D], bf16, kind="ExternalOutput")

    with tile.TileContext(nc) as tc:
        with (
            tc.tile_pool(name="const", bufs=1) as const_pool,
            tc.tile_pool(name="wts", bufs=1) as wts_pool,
            tc.tile_pool(name="big", bufs=1) as big_pool,
        ):
            # Input DMA is the kernel lead-in and every ns of it delays the
            # PE's first matmul, so it is split across both HWDGE queues
            # (SP="sync" and ACT="scalar") in consumption order: wv first
            # (the V projection runs first), then the x^T XBAR-transpose
            # half-slabs interleaved with wq/wk, and the attention-only
            # constants (tri, wo) last.
            tri_sb = const_pool.tile([128, 128], bf16, name="tri_sb", tag="tri_sb")
            wo_sb = []
            for g in range(2):
                t = const_pool.tile([128, D], bf16, name=f"wo{g}", tag=f"wo{g}")
                wo_sb.append(t)
            wq_sb, wk_sb, wv_sb = [], [], []
            for nm, lst in (("wq", wq_sb), ("wk", wk_sb), ("wv", wv_sb)):
                for r in range(KT):
                    t = wts_pool.tile(
                        [128, HPC * DH], bf16, name=f"{nm}{r}", tag=f"{nm}{r}"
                    )
                    lst.append(t)
            xT = []
            for r in range(KT):
                t = big_pool.tile([128, N], bf16, name=f"xT{r}", tag=f"xT{r}")
                xT.append(t)

            # Everything goes on ONE DMA queue: concurrent traffic on the
            # second HWDGE queue poisons the DMA cadence (6us per transfer
            # instead of 1.3us) and concurrent XBAR transposes on two queues
            # corrupt each other. Order is consumption order, so the PE's
            # first matmul only waits for wv + the half0 transposes.
            for r in range(KT):
                nc.sync.dma_start(out=wv_sb[r], in_=wv[128 * r : 128 * (r + 1), :])
            for r in range(KT):
                nc.sync.dma_start_transpose(
                    out=xT[r][:, 0:1024], in_=xb[0:1024, 128 * r : 128 * (r + 1)]
                )
            for r in range(KT):
                nc.sync.dma_start(out=wq_sb[r], in_=wq[128 * r : 128 * (r + 1), :])
            for r in range(KT):
                nc.sync.dma_start(out=wk_sb[r], in_=wk[128 * r : 128 * (r + 1), :])
            for r in range(KT):
                nc.sync.dma_start_transpose(
                    out=xT[r][:, 1024:2048],
                    in_=xb[1024:2048, 128 * r : 128 * (r + 1)],
                )
            nc.sync.dma_start(out=tri_sb, in_=tri[:, :])
            for g in range(2):
                nc.sync.dma_start(
                    out=wo_sb[g], in_=wo[128 * g : 128 * (g + 1), :]
                )

            with (
                tc.tile_pool(name="pj", bufs=2, space="PSUM") as pj_pool,
                tc.tile_pool(name="osb", bufs=3) as osb_pool,
            ):
                qT, kT_ = [], []
                for g in range(2):
                    tq = big_pool.tile([128, N], bf16, name=f"qT{g}", tag=f"qT{g}")
                    tk = big_pool.tile([128, N], bf16, name=f"kT{g}", tag=f"kT{g}")
                    qT.append(tq)
                    kT_.append(tk)
                v_all = big_pool.tile(
                    [128, NB * HPC * VW], bf16, name="v_all", tag="v_all"
                )
                # ones columns for the row-sum trick: fill the whole tile with
                # 1.0; the projection copies below overwrite the data columns
                nc.vector.memset(v_all, 1.0)


                def _copy(eng, out, in_):
                    if eng == "scalar":
                        nc.scalar.copy(out=out, in_=in_)
                    else:
                        getattr(nc, eng).tensor_copy(out=out, in_=in_)

                def vproj_stream(nbs):
                    for nb in nbs:
                        pv = pj_pool.tile([128, HPC * DH], f32, name="pv", tag="pj")
                        for r in range(KT):
                            nc.tensor.matmul(
                                pv,
                                lhsT=xT[r][:, 128 * nb : 128 * (nb + 1)],
                                rhs=wv_sb[r],
                                start=(r == 0),
                                stop=(r == KT - 1),
                            )
                        base = nb * HPC * VW
                        for h in range(HPC):
                            _copy(
                                ("vector", "scalar", "scalar", "vector")[h],
                                v_all[:, base + VW * h : base + VW * h + DH],
                                pv[:, DH * h : DH * (h + 1)],
                            )
                        yield

                def qkproj_stream(g, copy_engines=("any",)):
                    ci = 0
                    for s4 in range(4):
                        sl = slice(512 * s4, 512 * (s4 + 1))
                        for w_sb, dst in ((wq_sb, qT[g]), (wk_sb, kT_[g])):
                            pq = pj_pool.tile([128, 512], f32, name="pq", tag="pj")
                            for r in range(KT):
                                nc.tensor.matmul(
                                    pq,
                                    lhsT=w_sb[r][:, 128 * g : 128 * (g + 1)],
                                    rhs=xT[r][:, sl],
                                    start=(r == 0),
                                    stop=(r == KT - 1),
                                )
                            _copy(copy_engines[ci % len(copy_engines)], dst[:, sl], pq)
                            ci += 1
                            yield

                # ---------------- phase B: projections ----------------
                # emission order tracks the DMA half-slab arrival order; only
                # what attention head 0 cp0 needs (V nb0..7 + all of qk g0)
                # runs here — the rest of V and all of g1 fill PE slack
                # inside the attention loop.
                for _ in vproj_stream(range(8)):
                    pass
                for _ in qkproj_stream(0, copy_engines=("vector", "scalar")):
                    pass

                # ---------------- phase C: attention with interleaved
                # ---------------- g1 projections and output projection ------
                OT = []
                for g in range(2):
                    t = big_pool.tile([128, N], bf16, name=f"OT{g}", tag=f"OT{g}")
                    OT.append(t)

                def outproj_stream(nbs, copy_engines):
                    ci = 0
                    for nb in nbs:
                        nsl = slice(128 * nb, 128 * (nb + 1))
                        for s in range(2):
                            po = pj_pool.tile([128, 512], f32, name="po", tag="pj")
                            for g in range(2):
                                nc.tensor.matmul(
                                    po,
                                    lhsT=OT[g][:, nsl],
                                    rhs=wo_sb[g][:, 512 * s : 512 * (s + 1)],
                                    start=(g == 0),
                                    stop=(g == 1),
                                )
                            ob = osb_pool.tile([128, 512], bf16, name="ob", tag="osb")
                            _copy(copy_engines[ci % len(copy_engines)], ob, po)
                            ci += 1
                            nc.sync.dma_start(
                                out=outp[nsl, 512 * s : 512 * (s + 1)], in_=ob
                            )
                            yield

                with (
                    tc.tile_pool(name="pS", bufs=2, space="PSUM") as pS_pool,
                    tc.tile_pool(name="pO", bufs=1, space="PSUM") as pO_pool,
                    tc.tile_pool(name="att", bufs=4) as att_pool,
                ):
                    # A@V emission lags the QK/exp emission by DELAY jb-steps
                    # so the in-order PE never stalls on the ACT exp; the
                    # PE-dense projection streams above are pulled in between
                    # attention steps to fill the remaining PE idle time.
                    DELAY = 2
                    pend = []

                    def drain(n):
                        while len(pend) > n:
                            pend.pop(0)()

                    def attention_stream():
                        for h in range(HPC):
                            g, row = h // 2, 64 * (h % 2)
                            for cp in range(2):
                                jd, jmax = 8 * cp, 8 * (cp + 1)
                                pO = pO_pool.tile(
                                    [128, IC], f32, name=f"pO{h}", tag="pO"
                                )
                                for jb in range(jmax):
                                    rel = jb - jd
                                    o = 128 * rel if rel > 0 else 0
                                    jsl = slice(128 * jb, 128 * (jb + 1))
                                    pS = pS_pool.tile(
                                        [128, IC], f32, name="pS", tag="pS"
                                    )
                                    pexp = att_pool.tile(
                                        [128, IC], bf16, name="pexp", tag="pexp"
                                    )
                                    # S^T = kT_j^T . qT_i in 512-wide segments
                                    # trimmed exactly to the causal offset
                                    for s in range(2):
                                        a = max(o, 512 * s)
                                        if a >= 512 * (s + 1):
                                            continue
                                        nc.tensor.matmul(
                                            pS[:, a : 512 * (s + 1)],
                                            lhsT=kT_[g][row : row + 64, jsl],
                                            rhs=qT[g][
                                                row : row + 64,
                                                IC * cp + a : IC * cp + 512 * (s + 1),
                                            ],
                                            start=True,
                                            stop=True,
                                        )
                                    nc.scalar.activation(
                                        out=pexp[:, o:IC],
                                        in_=pS[:, o:IC],
                                        func=EXP,
                                        scale=SCALE,
                                    )
                                    if rel >= 0:
                                        nc.vector.tensor_mul(
                                            pexp[:, o : o + 128],
                                            pexp[:, o : o + 128],
                                            tri_sb,
                                        )

                                    def av_unit(
                                        h=h, jb=jb, o=o, jd=jd, jmax=jmax,
                                        pO=pO, pexp=pexp,
                                    ):
                                        # seg1 first (never overlaps the
                                        # masked triangle); exact causal trim
                                        for s in (1, 0):
                                            hi = 512 * (s + 1)
                                            lo = max(o, 512 * s)
                                            if lo >= hi:
                                                continue
                                            vsl = slice(
                                                jb * HPC * VW + VW * h,
                                                jb * HPC * VW + VW * (h + 1),
                                            )
                                            nc.tensor.matmul(
                                                pO[:, lo:hi],
                                                lhsT=v_all[:, vsl],
                                                rhs=pexp[:, lo:hi],
                                                start=(jb == 0),
                                                stop=(
                                                    jb
                                                    == (jd + 3 if s == 0 else jmax - 1)
                                                ),
                                                skip_group_check=True,
                                            )

                                    pend.append(av_unit)
                                    drain(DELAY)
                                    yield (h, cp, jb)

                                # Normalize: split the O^T psum into two
                                # partition-0-aligned SBUF tiles (O rows on
                                # ACT, sum rows on DVE — reciprocal_approx_
                                # fast's custom ucode can't take a partition
                                # offset), one approx-reciprocal over the
                                # whole sums tile, then the broadcast
                                # multiplies in 256-col chunks spread through
                                # the pend queue.
                                pOcO = att_pool.tile(
                                    [64, IC], f32, name="pOcO", tag="pOcO", bufs=2
                                )
                                pOcS = att_pool.tile(
                                    [64, IC], f32, name="pOcS", tag="pOcS", bufs=2
                                )
                                rec = att_pool.tile(
                                    [64, IC], f32, name="rec", tag="rec", bufs=2
                                )

                                def copy_unit(pO=pO, pOcO=pOcO, pOcS=pOcS, rec=rec):
                                    nc.scalar.copy(out=pOcO, in_=pO[0:64, :])
                                    nc.vector.tensor_copy(
                                        out=pOcS, in_=pO[64:128, :]
                                    )
                                    nc.vector.reciprocal_approx_fast(
                                        out=rec, in_=pOcS
                                    )

                                pend.append(copy_unit)
                                for c4 in range(4):
                                    def norm_chunk(
                                        g=g, row=row, cp=cp, pOcO=pOcO,
                                        rec=rec, c4=c4,
                                    ):
                                        cs = slice(256 * c4, 256 * (c4 + 1))
                                        nc.vector.tensor_mul(
                                            OT[g][
                                                row : row + 64,
                                                IC * cp + 256 * c4 : IC * cp
                                                + 256 * (c4 + 1),
                                            ],
                                            pOcO[:, cs],
                                            rec[:, cs],
                                        )

                                    pend.append(norm_chunk)

                    att = attention_stream()
                    vfill = vproj_stream(range(8, 16))
                    g1fill = qkproj_stream(1, copy_engines=("vector",))
                    out0fill = outproj_stream(range(8), ("vector",))
                    v_done = False
                    g1_done = False
                    steps = 0
                    for h, cp, jb in att:
                        steps += 1
                        if not v_done:
                            v_done = next(vfill, "end") == "end"
                        elif not g1_done and steps % 3 == 0:
                            g1_done = next(g1fill, "end") == "end"
                        if h == 2 and not g1_done:
                            # g1 attention needs qT[1]/kT[1] complete
                            for _ in g1fill:
                                pass
                            g1_done = True
                        # nb 0..7 of the output projection read cp0 columns of
                        # OT; head 3's cp0 normalize leaves the pend queue at
                        # jb==DELAY of cp1, so only start pulling after that
                        if h == 3 and cp == 1 and jb > DELAY:
                            next(out0fill, None)
                            next(out0fill, None)
                    drain(0)
                    for _ in out0fill:
                        pass
                    for _ in outproj_stream(range(8, 16), ("scalar", "vector")):
                        pass

    nc.compile()
    return nc


def kernel(x, mask, Wq, Wkv, Wout, b_out):
    global _last_results
    import ml_dtypes
    from concourse.bass_utils import run_bass_kernel_spmd

    bf = ml_dtypes.bfloat16
    x = np.asarray(x, dtype=np.float32).astype(bf)
    Wq = np.asarray(Wq, dtype=np.float32).astype(bf)
    Wkv = np.asarray(Wkv, dtype=np.float32).astype(bf)
    Wout = np.asarray(Wout, dtype=np.float32).astype(bf)
    b_out = np.asarray(b_out, dtype=np.float32)

    if "nc" not in _cached:
        _cached["nc"] = _build_program()
    nc = _cached["nc"]

    jj, ii = np.mgrid[0:128, 0:128]
    tri = (jj <= ii).astype(bf)

    in_maps = []
    for c in range(NCORES):
        b = c // 4
        h0 = HPC * (c % 4)
        in_maps.append(
            {
                "xb": np.ascontiguousarray(x[b]),
                "wq": np.ascontiguousarray(Wq[:, DH * h0 : DH * (h0 + HPC)]),
                "wk": np.ascontiguousarray(Wkv[:, DH * h0 : DH * (h0 + HPC)]),
                "wv": np.ascontiguousarray(
                    Wkv[:, D + DH * h0 : D + DH * (h0 + HPC)]
                ),
                "wo": np.ascontiguousarray(Wout[DH * h0 : DH * (h0 + HPC), :]),
                "tri": tri,
            }
        )

    res = run_bass_kernel_spmd(
        nc,
        in_maps,
        core_ids=list(range(NCORES)),
        trace=bool(int(os.environ.get("KERNEL_TRACE", "0"))),
    )
    _last_results = res
    parts = [r["outp"] for r in res.results]
    out = np.empty((B, N, D), dtype=np.float32)
    for b in range(B):
        acc = np.asarray(parts[4 * b], dtype=np.float32)
        for c in range(4 * b + 1, 4 * b + 4):
            acc += np.asarray(parts[c], dtype=np.float32)
        out[b] = acc + b_out[None, :]
    return out


# revision 19
# speedup vs baseline: 1.3425x; 1.0349x over previous
"""Causal multi-head attention kernel for 8 trn2 NeuronCores.

Problem: x[2,2048,1024], 16 heads of dim 64, causal softmax(q k^T / sqrt(1024)) v,
then output projection. Sharding: data-parallel over batch (4 cores per batch),
tensor-parallel over heads (4 heads per core). Each core produces a partial
output (its heads' contribution through Wout); the host sums the 4 partials per
batch and adds b_out.

Datapath is bf16 (inputs cast on host) with fp32 PSUM accumulation; the
rel-err budget is 2e-2 and bf16 keeps the end-to-end error ~5e-3. bf16 buys:
full PE rate at any free size (no >=256 fp32r padding), DMA-engine XBAR
transpose of x (the whole PE transpose phase is gone), half the DMA bytes,
and 2x DVE modes on the mask multiplies.

Per-core device program (SPMD, per-core data arrives via input tensors):
  1. xT [d on partitions, n free] arrives directly via dma_start_transpose
     (16 half-slab XBAR transposes), overlapped with weight DMAs.
  2. Projections: qT/kT [dh on partitions, n free] (2-head groups of 128
     partitions), v natural [n on partitions] augmented with a ones column so
     the attention matmul also produces softmax row-sums.
  3. Per head, per 1024-wide i-chunk: S^T[j-block, i] = kT_j^T . qT_i on PE,
     exp((1/32) S) on ACT straight out of PSUM (logits are bounded, no
     max-subtraction needed) writing bf16, triangular mask multiply only on
     the diagonal 128x128 block, then O^T[dh+1, i] += V_aug^T . P^T
     accumulated over j-blocks in PSUM. Block-causality skips all j>i blocks,
     and segments are trimmed exactly to the causal offset.
  4. Normalize by the row-sums (reciprocal_approx_fast + broadcast multiply),
     output projection with two 2-head pairs packed to a full K=128
     contraction.
"""

import os

import numpy as np

B, N, D, H = 2, 2048, 1024, 16
DH = D // H  # 64
SCALE = float(D) ** -0.5
NCORES = 8
HPC = 4  # heads per core
IC = 1024  # i-chunk width in attention phase
NB = N // 128  # 16 j/n blocks
KT = D // 128  # 8 contraction tiles
# v columns per head: 64 data cols + 64 ones cols. The ones columns make the
# attention matmul write the softmax row-sum replicated on PSUM partitions
# 64..127, so normalization is a plain elementwise reciprocal+multiply
# (partitions 0..63 / partitions 64..127) with no partition-broadcast needed.
VW = 2 * DH  # 128

_cached = {}
_last_results = None


def _build_program():
    import concourse.bacc as bacc
    import concourse.mybir as mybir
    import concourse.tile as tile

    f32 = mybir.dt.float32
    bf16 = mybir.dt.bfloat16
    EXP = mybir.ActivationFunctionType.Exp

    nc = bacc.Bacc()

    xb = nc.dram_tensor("xb", [N, D], bf16, kind="ExternalInput")
    wq = nc.dram_tensor("wq", [D, HPC * DH], bf16, kind="ExternalInput")
    wk = nc.dram_tensor("wk", [D, HPC * DH], bf16, kind="ExternalInput")
    wv = nc.dram_tensor("wv", [D, HPC * DH], bf16, kind="ExternalInput")
    wo = nc.dram_tensor("wo", [HPC * DH, D], bf16, kind="ExternalInput")
    tri = nc.dram_tensor("tri", [128, 128], bf16, kind="ExternalInput")
    outp = nc.dram_tensor("outp", [N, D], bf16, kind="ExternalOutput")

    with tile.TileContext(nc) as tc:
        with (
            tc.tile_pool(name="const", bufs=1) as const_pool,
            tc.tile_pool(name="wts", bufs=1) as wts_pool,
            tc.tile_pool(name="big", bufs=1) as big_pool,
        ):
            # Input DMA is the kernel lead-in and every ns of it delays the
            # PE's first matmul, so it is split across both HWDGE queues
            # (SP="sync" and ACT="scalar") in consumption order: wv first
            # (the V projection runs first), then the x^T XBAR-transpose
            # half-slabs interleaved with wq/wk, and the attention-only
            # constants (tri, wo) last.
            tri_sb = const_pool.tile([128, 128], bf16, name="tri_sb", tag="tri_sb")
            wo_all = const_pool.tile([128, 2 * D], bf16, name="wo_all", tag="wo_all")
            wo_sb = [wo_all[:, D * g : D * (g + 1)] for g in range(2)]
            # each [D, 256] weight loads as ONE DMA into [128, 8*256] (the 8
            # 128-row D-tiles side by side) — 1 queue slot instead of 8
            wq_sb, wk_sb, wv_sb = [], [], []
            w_alls = {}
            for nm, lst in (("wq", wq_sb), ("wk", wk_sb), ("wv", wv_sb)):
                t = wts_pool.tile(
                    [128, KT * HPC * DH], bf16, name=f"{nm}_all", tag=f"{nm}_all"
                )
                w_alls[nm] = t
                for r in range(KT):
                    lst.append(t[:, HPC * DH * r : HPC * DH * (r + 1)])
            xT = []
            for r in range(KT):
                t = big_pool.tile([128, N], bf16, name=f"xT{r}", tag=f"xT{r}")
                xT.append(t)

            # Everything goes on ONE DMA queue: concurrent traffic on the
            # second HWDGE queue poisons the DMA cadence (6us per transfer
            # instead of 1.3us) and concurrent XBAR transposes on two queues
            # corrupt each other. Order is consumption order, so the PE's
            # first matmul only waits for wv + the half0 transposes.
            nc.sync.dma_start(
                out=w_alls["wv"].rearrange("p (r c) -> p r c", r=KT),
                in_=wv[:, :].rearrange("(r p) c -> p r c", r=KT),
            )
            for r in range(KT):
                nc.sync.dma_start_transpose(
                    out=xT[r][:, 0:1024], in_=xb[0:1024, 128 * r : 128 * (r + 1)]
                )
            nc.sync.dma_start(
                out=w_alls["wq"].rearrange("p (r c) -> p r c", r=KT),
                in_=wq[:, :].rearrange("(r p) c -> p r c", r=KT),
            )
            nc.sync.dma_start(
                out=w_alls["wk"].rearrange("p (r c) -> p r c", r=KT),
                in_=wk[:, :].rearrange("(r p) c -> p r c", r=KT),
            )
            for r in range(KT):
                nc.sync.dma_start_transpose(
                    out=xT[r][:, 1024:2048],
                    in_=xb[1024:2048, 128 * r : 128 * (r + 1)],
                )
            nc.sync.dma_start(out=tri_sb, in_=tri[:, :])
            nc.sync.dma_start(
                out=wo_all.rearrange("p (g c) -> p g c", g=2),
                in_=wo[:, :].rearrange("(g p) c -> p g c", g=2),
            )

            with (
                tc.tile_pool(name="pj", bufs=2, space="PSUM") as pj_pool,
                tc.tile_pool(name="osb", bufs=3) as osb_pool,
            ):
                qT, kT_ = [], []
                for g in range(2):
                    tq = big_pool.tile([128, N], bf16, name=f"qT{g}", tag=f"qT{g}")
                    tk = big_pool.tile([128, N], bf16, name=f"kT{g}", tag=f"kT{g}")
                    qT.append(tq)
                    kT_.append(tk)
                v_all = big_pool.tile(
                    [128, NB * HPC * VW], bf16, name="v_all", tag="v_all"
                )
                # ones columns for the row-sum trick: fill the whole tile with
                # 1.0; the projection copies below overwrite the data columns
                nc.vector.memset(v_all, 1.0)


                def _copy(eng, out, in_):
                    if eng == "scalar":
                        nc.scalar.copy(out=out, in_=in_)
                    else:
                        getattr(nc, eng).tensor_copy(out=out, in_=in_)

                def vproj_stream(nbs):
                    for nb in nbs:
                        pv = pj_pool.tile([128, HPC * DH], f32, name="pv", tag="pj")
                        for r in range(KT):
                            nc.tensor.matmul(
                                pv,
                                lhsT=xT[r][:, 128 * nb : 128 * (nb + 1)],
                                rhs=wv_sb[r],
                                start=(r == 0),
                                stop=(r == KT - 1),
                            )
                        base = nb * HPC * VW
                        for h in range(HPC):
                            _copy(
                                ("vector", "scalar", "scalar", "vector")[h],
                                v_all[:, base + VW * h : base + VW * h + DH],
                                pv[:, DH * h : DH * (h + 1)],
                            )
                        yield

                def qkproj_stream(g, copy_engines=("any",)):
                    ci = 0
                    for s4 in range(4):
                        sl = slice(512 * s4, 512 * (s4 + 1))
                        for w_sb, dst in ((wq_sb, qT[g]), (wk_sb, kT_[g])):
                            pq = pj_pool.tile([128, 512], f32, name="pq", tag="pj")
                            for r in range(KT):
                                nc.tensor.matmul(
                                    pq,
                                    lhsT=w_sb[r][:, 128 * g : 128 * (g + 1)],
                                    rhs=xT[r][:, sl],
                                    start=(r == 0),
                                    stop=(r == KT - 1),
                                )
                            _copy(copy_engines[ci % len(copy_engines)], dst[:, sl], pq)
                            ci += 1
                            yield

                # ---------------- phase B: projections ----------------
                # emission order tracks the DMA half-slab arrival order; only
                # what attention head 0 cp0 needs (V nb0..7 + all of qk g0)
                # runs here — the rest of V and all of g1 fill PE slack
                # inside the attention loop.
                for _ in vproj_stream(range(8)):
                    pass
                for _ in qkproj_stream(0, copy_engines=("vector", "scalar")):
                    pass

                # ---------------- phase C: attention with interleaved
                # ---------------- g1 projections and output projection ------
                OT = []
                for g in range(2):
                    t = big_pool.tile([128, N], bf16, name=f"OT{g}", tag=f"OT{g}")
                    OT.append(t)

                def outproj_stream(nbs, copy_engines):
                    ci = 0
                    for nb in nbs:
                        nsl = slice(128 * nb, 128 * (nb + 1))
                        for s in range(2):
                            po = pj_pool.tile([128, 512], f32, name="po", tag="pj")
                            for g in range(2):
                                nc.tensor.matmul(
                                    po,
                                    lhsT=OT[g][:, nsl],
                                    rhs=wo_sb[g][:, 512 * s : 512 * (s + 1)],
                                    start=(g == 0),
                                    stop=(g == 1),
                                )
                            ob = osb_pool.tile([128, 512], bf16, name="ob", tag="osb")
                            _copy(copy_engines[ci % len(copy_engines)], ob, po)
                            ci += 1
                            nc.sync.dma_start(
                                out=outp[nsl, 512 * s : 512 * (s + 1)], in_=ob
                            )
                            yield

                with (
                    tc.tile_pool(name="pS", bufs=2, space="PSUM") as pS_pool,
                    tc.tile_pool(name="pO", bufs=1, space="PSUM") as pO_pool,
                    tc.tile_pool(name="att", bufs=4) as att_pool,
                ):
                    # A@V emission lags the QK/exp emission by DELAY jb-steps
                    # so the in-order PE never stalls on the ACT exp; the
                    # PE-dense projection streams above are pulled in between
                    # attention steps to fill the remaining PE idle time.
                    DELAY = 3
                    pend = []

                    def drain(n):
                        while len(pend) > n:
                            pend.pop(0)()

                    def attention_stream():
                        for h in range(HPC):
                            g, row = h // 2, 64 * (h % 2)
                            for cp in range(2):
                                jd, jmax = 8 * cp, 8 * (cp + 1)
                                pO = pO_pool.tile(
                                    [128, IC], f32, name=f"pO{h}", tag="pO"
                                )
                                for jb in range(jmax):
                                    rel = jb - jd
                                    o = 128 * rel if rel > 0 else 0
                                    jsl = slice(128 * jb, 128 * (jb + 1))
                                    pS = pS_pool.tile(
                                        [128, IC], f32, name="pS", tag="pS"
                                    )
                                    pexp = att_pool.tile(
                                        [128, IC], bf16, name="pexp", tag="pexp",
                                        bufs=5,
                                    )
                                    # S^T = kT_j^T . qT_i in 512-wide segments
                                    # trimmed exactly to the causal offset
                                    for s in range(2):
                                        a = max(o, 512 * s)
                                        if a >= 512 * (s + 1):
                                            continue
                                        nc.tensor.matmul(
                                            pS[:, a : 512 * (s + 1)],
                                            lhsT=kT_[g][row : row + 64, jsl],
                                            rhs=qT[g][
                                                row : row + 64,
                                                IC * cp + a : IC * cp + 512 * (s + 1),
                                            ],
                                            start=True,
                                            stop=True,
                                        )
                                    nc.scalar.activation(
                                        out=pexp[:, o:IC],
                                        in_=pS[:, o:IC],
                                        func=EXP,
                                        scale=SCALE,
                                    )
                                    if rel >= 0:
                                        nc.vector.tensor_mul(
                                            pexp[:, o : o + 128],
                                            pexp[:, o : o + 128],
                                            tri_sb,
                                        )

                                    def av_unit(
                                        h=h, jb=jb, o=o, jd=jd, jmax=jmax,
                                        pO=pO, pexp=pexp,
                                    ):
                                        # seg1 first (never overlaps the
                                        # masked triangle); exact causal trim
                                        for s in (1, 0):
                                            hi = 512 * (s + 1)
                                            lo = max(o, 512 * s)
                                            if lo >= hi:
                                                continue
                                            vsl = slice(
                                                jb * HPC * VW + VW * h,
                                                jb * HPC * VW + VW * (h + 1),
                                            )
                                            nc.tensor.matmul(
                                                pO[:, lo:hi],
                                                lhsT=v_all[:, vsl],
                                                rhs=pexp[:, lo:hi],
                                                start=(jb == 0),
                                                stop=(
                                                    jb
                                                    == (jd + 3 if s == 0 else jmax - 1)
                                                ),
                                                skip_group_check=True,
                                            )

                                    pend.append(av_unit)
                                    drain(DELAY)
                                    yield (h, cp, jb)

                                # Normalize: split the O^T psum into two
                                # partition-0-aligned SBUF tiles (O rows on
                                # ACT, sum rows on DVE — reciprocal_approx_
                                # fast's custom ucode can't take a partition
                                # offset), one approx-reciprocal over the
                                # whole sums tile, then the broadcast
                                # multiplies in 256-col chunks spread through
                                # the pend queue.
                                pOcO = att_pool.tile(
                                    [64, IC], f32, name="pOcO", tag="pOcO", bufs=2
                                )
                                pOcS = att_pool.tile(
                                    [64, IC], f32, name="pOcS", tag="pOcS", bufs=2
                                )
                                rec = att_pool.tile(
                                    [64, IC], f32, name="rec", tag="rec", bufs=2
                                )

                                def copy_unit(pO=pO, pOcO=pOcO, pOcS=pOcS, rec=rec):
                                    nc.scalar.copy(out=pOcO, in_=pO[0:64, :])
                                    nc.vector.tensor_copy(
                                        out=pOcS, in_=pO[64:128, :]
                                    )
                                    nc.vector.reciprocal_approx_fast(
                                        out=rec, in_=pOcS
                                    )

                                pend.append(copy_unit)
                                for c4 in range(4):
                                    def norm_chunk(
                                        g=g, row=row, cp=cp, pOcO=pOcO,
                                        rec=rec, c4=c4,
                                    ):
                                        cs = slice(256 * c4, 256 * (c4 + 1))
                                        nc.vector.tensor_mul(
                                            OT[g][
                                                row : row + 64,
                                                IC * cp + 256 * c4 : IC * cp
                                                + 256 * (c4 + 1),
                                            ],
                                            pOcO[:, cs],
                                            rec[:, cs],
                                        )

                                    pend.append(norm_chunk)

                    att = attention_stream()
                    vfill = vproj_stream(range(8, 16))
                    g1fill = qkproj_stream(1, copy_engines=("vector",))
                    out0fill = outproj_stream(range(8), ("vector",))
                    v_done = False
                    g1_done = False
                    steps = 0
                    for h, cp, jb in att:
                        steps += 1
                        if not v_done:
                            v_done = next(vfill, "end") == "end"
                        elif not g1_done and steps % 3 == 0:
                            g1_done = next(g1fill, "end") == "end"
                        if h == 2 and not g1_done:
                            # g1 attention needs qT[1]/kT[1] complete
                            for _ in g1fill:
                                pass
                            g1_done = True
                        # nb 0..7 of the output projection read cp0 columns of
                        # OT; head 3's cp0 normalize leaves the pend queue at
                        # jb==DELAY of cp1, so only start pulling after that
                        if h == 3 and cp == 1 and jb > DELAY:
                            next(out0fill, None)
                            next(out0fill, None)
                    # drain the tail interleaved with the final output
                    # blocks: norm chunk c4 of (h3, cp1) unlocks output
                    # columns 256*c4..256*(c4+1), i.e. nb 8+2*c4, 9+2*c4
                    outfin = outproj_stream(range(8, 16), ("scalar", "vector"))
                    while pend:
                        pend.pop(0)()
                        if len(pend) < 4:
                            for _ in range(4):
                                next(outfin, None)
                    for _ in out0fill:
                        pass
                    for _ in outfin:
                        pass

    nc.compile()
    return nc


def kernel(x, mask, Wq, Wkv, Wout, b_out):
    global _last_results
    import ml_dtypes
    from concourse.bass_utils import run_bass_kernel_spmd

    bf = ml_dtypes.bfloat16
    x = np.asarray(x, dtype=np.float32).astype(bf)
    Wq = np.asarray(Wq, dtype=np.float32).astype(bf)
    Wkv = np.asarray(Wkv, dtype=np.float32).astype(bf)
    Wout = np.asarray(Wout, dtype=np.float32).astype(bf)
    b_out = np.asarray(b_out, dtype=np.float32)

    if "nc" not in _cached:
        _cached["nc"] = _build_program()
    nc = _cached["nc"]

    jj, ii = np.mgrid[0:128, 0:128]
    tri = (jj <= ii).astype(bf)

    in_maps = []
    for c in range(NCORES):
        b = c // 4
        h0 = HPC * (c % 4)
        in_maps.append(
            {
                "xb": np.ascontiguousarray(x[b]),
                "wq": np.ascontiguousarray(Wq[:, DH * h0 : DH * (h0 + HPC)]),
                "wk": np.ascontiguousarray(Wkv[:, DH * h0 : DH * (h0 + HPC)]),
                "wv": np.ascontiguousarray(
                    Wkv[:, D + DH * h0 : D + DH * (h0 + HPC)]
                ),
                "wo": np.ascontiguousarray(Wout[DH * h0 : DH * (h0 + HPC), :]),
                "tri": tri,
            }
        )

    res = run_bass_kernel_spmd(
        nc,
        in_maps,
        core_ids=list(range(NCORES)),
        trace=bool(int(os.environ.get("KERNEL_TRACE", "0"))),
    )
    _last_results = res
    parts = [r["outp"] for r in res.results]
    out = np.empty((B, N, D), dtype=np.float32)
    for b in range(B):
        acc = np.asarray(parts[4 * b], dtype=np.float32)
        for c in range(4 * b + 1, 4 * b + 4):
            acc += np.asarray(parts[c], dtype=np.float32)
        out[b] = acc + b_out[None, :]
    return out


# revision 21
# speedup vs baseline: 1.3843x; 1.0312x over previous
"""Causal multi-head attention kernel for 8 trn2 NeuronCores.

Problem: x[2,2048,1024], 16 heads of dim 64, causal softmax(q k^T / sqrt(1024)) v,
then output projection. Sharding: data-parallel over batch (4 cores per batch),
tensor-parallel over heads (4 heads per core). Each core produces a partial
output (its heads' contribution through Wout); the host sums the 4 partials per
batch and adds b_out.

Datapath is bf16 (inputs cast on host) with fp32 PSUM accumulation; the
rel-err budget is 2e-2 and bf16 keeps the end-to-end error ~5e-3. bf16 buys:
full PE rate at any free size (no >=256 fp32r padding), DMA-engine XBAR
transpose of x (the whole PE transpose phase is gone), half the DMA bytes,
and 2x DVE modes on the mask multiplies.

Per-core device program (SPMD, per-core data arrives via input tensors):
  1. xT [d on partitions, n free] arrives directly via dma_start_transpose
     (16 half-slab XBAR transposes), overlapped with weight DMAs.
  2. Projections: qT/kT [dh on partitions, n free] (2-head groups of 128
     partitions), v natural [n on partitions] augmented with a ones column so
     the attention matmul also produces softmax row-sums.
  3. Per head, per 1024-wide i-chunk: S^T[j-block, i] = kT_j^T . qT_i on PE,
     exp((1/32) S) on ACT straight out of PSUM (logits are bounded, no
     max-subtraction needed) writing bf16, triangular mask multiply only on
     the diagonal 128x128 block, then O^T[dh+1, i] += V_aug^T . P^T
     accumulated over j-blocks in PSUM. Block-causality skips all j>i blocks,
     and segments are trimmed exactly to the causal offset.
  4. Normalize by the row-sums (reciprocal_approx_fast + broadcast multiply),
     output projection with two 2-head pairs packed to a full K=128
     contraction.
"""

import os

import numpy as np

B, N, D, H = 2, 2048, 1024, 16
DH = D // H  # 64
SCALE = float(D) ** -0.5
NCORES = 8
HPC = 4  # heads per core
IC = 1024  # i-chunk width in attention phase
NB = N // 128  # 16 j/n blocks
KT = D // 128  # 8 contraction tiles
# v columns per head: 64 data cols + 64 ones cols. The ones columns make the
# attention matmul write the softmax row-sum replicated on PSUM partitions
# 64..127, so normalization is a plain elementwise reciprocal+multiply
# (partitions 0..63 / partitions 64..127) with no partition-broadcast needed.
VW = 2 * DH  # 128

_cached = {}
_last_results = None


def _build_program():
    import concourse.bacc as bacc
    import concourse.mybir as mybir
    import concourse.tile as tile

    f32 = mybir.dt.float32
    bf16 = mybir.dt.bfloat16
    EXP = mybir.ActivationFunctionType.Exp

    nc = bacc.Bacc()

    xb = nc.dram_tensor("xb", [N, D], bf16, kind="ExternalInput")
    wq = nc.dram_tensor("wq", [D, HPC * DH], bf16, kind="ExternalInput")
    wk = nc.dram_tensor("wk", [D, HPC * DH], bf16, kind="ExternalInput")
    wv = nc.dram_tensor("wv", [D, HPC * DH], bf16, kind="ExternalInput")
    wo = nc.dram_tensor("wo", [HPC * DH, D], bf16, kind="ExternalInput")
    tri = nc.dram_tensor("tri", [128, 128], bf16, kind="ExternalInput")
    outp = nc.dram_tensor("outp", [N, D], bf16, kind="ExternalOutput")

    with tile.TileContext(nc) as tc:
        with (
            tc.tile_pool(name="const", bufs=1) as const_pool,
            tc.tile_pool(name="wts", bufs=1) as wts_pool,
            tc.tile_pool(name="big", bufs=1) as big_pool,
        ):
            # Input DMA is the kernel lead-in and every ns of it delays the
            # PE's first matmul, so it is split across both HWDGE queues
            # (SP="sync" and ACT="scalar") in consumption order: wv first
            # (the V projection runs first), then the x^T XBAR-transpose
            # half-slabs interleaved with wq/wk, and the attention-only
            # constants (tri, wo) last.
            tri_sb = const_pool.tile([128, 128], bf16, name="tri_sb", tag="tri_sb")
            wo_all = const_pool.tile([128, 2 * D], bf16, name="wo_all", tag="wo_all")
            wo_sb = [wo_all[:, D * g : D * (g + 1)] for g in range(2)]
            # each [D, 256] weight loads as ONE DMA into [128, 8*256] (the 8
            # 128-row D-tiles side by side) — 1 queue slot instead of 8
            wq_sb, wk_sb, wv_sb = [], [], []
            w_alls = {}
            for nm, lst in (("wq", wq_sb), ("wk", wk_sb), ("wv", wv_sb)):
                t = wts_pool.tile(
                    [128, KT * HPC * DH], bf16, name=f"{nm}_all", tag=f"{nm}_all"
                )
                w_alls[nm] = t
                for r in range(KT):
                    lst.append(t[:, HPC * DH * r : HPC * DH * (r + 1)])
            xT = []
            for r in range(KT):
                t = big_pool.tile([128, N], bf16, name=f"xT{r}", tag=f"xT{r}")
                xT.append(t)

            # Everything goes on ONE DMA queue: concurrent traffic on the
            # second HWDGE queue poisons the DMA cadence (6us per transfer
            # instead of 1.3us) and concurrent XBAR transposes on two queues
            # corrupt each other. Order is consumption order, so the PE's
            # first matmul only waits for wv + the half0 transposes.
            nc.sync.dma_start(
                out=w_alls["wv"].rearrange("p (r c) -> p r c", r=KT),
                in_=wv[:, :].rearrange("(r p) c -> p r c", r=KT),
            )
            for r in range(KT):
                nc.sync.dma_start_transpose(
                    out=xT[r][:, 0:1024], in_=xb[0:1024, 128 * r : 128 * (r + 1)]
                )
            nc.sync.dma_start(
                out=w_alls["wq"].rearrange("p (r c) -> p r c", r=KT),
                in_=wq[:, :].rearrange("(r p) c -> p r c", r=KT),
            )
            nc.sync.dma_start(
                out=w_alls["wk"].rearrange("p (r c) -> p r c", r=KT),
                in_=wk[:, :].rearrange("(r p) c -> p r c", r=KT),
            )
            nc.sync.dma_start(out=tri_sb, in_=tri[:, :])
            for r in range(KT):
                nc.sync.dma_start_transpose(
                    out=xT[r][:, 1024:2048],
                    in_=xb[1024:2048, 128 * r : 128 * (r + 1)],
                )
            nc.sync.dma_start(
                out=wo_all.rearrange("p (g c) -> p g c", g=2),
                in_=wo[:, :].rearrange("(g p) c -> p g c", g=2),
            )

            with (
                tc.tile_pool(name="pj", bufs=2, space="PSUM") as pj_pool,
                tc.tile_pool(name="osb", bufs=3) as osb_pool,
            ):
                qT, kT_ = [], []
                for g in range(2):
                    tq = big_pool.tile([128, N], bf16, name=f"qT{g}", tag=f"qT{g}")
                    tk = big_pool.tile([128, N], bf16, name=f"kT{g}", tag=f"kT{g}")
                    qT.append(tq)
                    kT_.append(tk)
                v_all = big_pool.tile(
                    [128, NB * HPC * VW], bf16, name="v_all", tag="v_all"
                )
                # ones columns for the row-sum trick: fill the whole tile with
                # 1.0; the projection copies below overwrite the data columns
                nc.vector.memset(v_all, 1.0)


                def _copy(eng, out, in_):
                    if eng == "scalar":
                        nc.scalar.copy(out=out, in_=in_)
                    else:
                        getattr(nc, eng).tensor_copy(out=out, in_=in_)

                def vproj_stream(nbs):
                    for nb in nbs:
                        pv = pj_pool.tile([128, HPC * DH], f32, name="pv", tag="pj")
                        for r in range(KT):
                            nc.tensor.matmul(
                                pv,
                                lhsT=xT[r][:, 128 * nb : 128 * (nb + 1)],
                                rhs=wv_sb[r],
                                start=(r == 0),
                                stop=(r == KT - 1),
                            )
                        base = nb * HPC * VW
                        for h in range(HPC):
                            _copy(
                                ("vector", "scalar", "scalar", "vector")[h],
                                v_all[:, base + VW * h : base + VW * h + DH],
                                pv[:, DH * h : DH * (h + 1)],
                            )
                        yield

                def qkproj_stream(g, copy_engines=("any",)):
                    ci = 0
                    for s4 in range(4):
                        sl = slice(512 * s4, 512 * (s4 + 1))
                        for w_sb, dst in ((wq_sb, qT[g]), (wk_sb, kT_[g])):
                            pq = pj_pool.tile([128, 512], f32, name="pq", tag="pj")
                            for r in range(KT):
                                nc.tensor.matmul(
                                    pq,
                                    lhsT=w_sb[r][:, 128 * g : 128 * (g + 1)],
                                    rhs=xT[r][:, sl],
                                    start=(r == 0),
                                    stop=(r == KT - 1),
                                )
                            _copy(copy_engines[ci % len(copy_engines)], dst[:, sl], pq)
                            ci += 1
                            yield

                # ---------------- phase B: projections ----------------
                # emission order tracks the DMA half-slab arrival order; only
                # what attention head 0 cp0 needs (V nb0..7 + all of qk g0)
                # runs here — the rest of V and all of g1 fill PE slack
                # inside the attention loop.
                for _ in vproj_stream(range(8)):
                    pass
                g0fill = qkproj_stream(0, copy_engines=("vector", "scalar"))
                for _ in range(4):
                    next(g0fill)

                # ---------------- phase C: attention with interleaved
                # ---------------- g1 projections and output projection ------
                OT = []
                for g in range(2):
                    t = big_pool.tile([128, N], bf16, name=f"OT{g}", tag=f"OT{g}")
                    OT.append(t)

                def outproj_stream(nbs, copy_engines):
                    ci = 0
                    for nb in nbs:
                        nsl = slice(128 * nb, 128 * (nb + 1))
                        ob = osb_pool.tile([128, D], bf16, name="ob", tag="osb")
                        for s in range(2):
                            po = pj_pool.tile([128, 512], f32, name="po", tag="pj")
                            for g in range(2):
                                nc.tensor.matmul(
                                    po,
                                    lhsT=OT[g][:, nsl],
                                    rhs=wo_sb[g][:, 512 * s : 512 * (s + 1)],
                                    start=(g == 0),
                                    stop=(g == 1),
                                )
                            _copy(
                                copy_engines[ci % len(copy_engines)],
                                ob[:, 512 * s : 512 * (s + 1)],
                                po,
                            )
                            ci += 1
                            if s == 1:
                                nc.sync.dma_start(out=outp[nsl, :], in_=ob)
                            yield

                with (
                    tc.tile_pool(name="pS", bufs=2, space="PSUM") as pS_pool,
                    tc.tile_pool(name="pO", bufs=1, space="PSUM") as pO_pool,
                    tc.tile_pool(name="att", bufs=4) as att_pool,
                ):
                    # A@V emission lags the QK/exp emission by DELAY jb-steps
                    # so the in-order PE never stalls on the ACT exp; the
                    # PE-dense projection streams above are pulled in between
                    # attention steps to fill the remaining PE idle time.
                    DELAY = 3
                    pend = []

                    def drain(n):
                        while len(pend) > n:
                            pend.pop(0)()

                    def attention_stream():
                        for h in range(HPC):
                            g, row = h // 2, 64 * (h % 2)
                            for cp in range(2):
                                jd, jmax = 8 * cp, 8 * (cp + 1)
                                pO = pO_pool.tile(
                                    [128, IC], f32, name=f"pO{h}", tag="pO"
                                )
                                for jb in range(jmax):
                                    rel = jb - jd
                                    o = 128 * rel if rel > 0 else 0
                                    jsl = slice(128 * jb, 128 * (jb + 1))
                                    pS = pS_pool.tile(
                                        [128, IC], f32, name="pS", tag="pS"
                                    )
                                    pexp = att_pool.tile(
                                        [128, IC], bf16, name="pexp", tag="pexp",
                                        bufs=5,
                                    )
                                    # S^T = kT_j^T . qT_i in 512-wide segments
                                    # trimmed exactly to the causal offset
                                    for s in range(2):
                                        a = max(o, 512 * s)
                                        if a >= 512 * (s + 1):
                                            continue
                                        nc.tensor.matmul(
                                            pS[:, a : 512 * (s + 1)],
                                            lhsT=kT_[g][row : row + 64, jsl],
                                            rhs=qT[g][
                                                row : row + 64,
                                                IC * cp + a : IC * cp + 512 * (s + 1),
                                            ],
                                            start=True,
                                            stop=True,
                                        )
                                    nc.scalar.activation(
                                        out=pexp[:, o:IC],
                                        in_=pS[:, o:IC],
                                        func=EXP,
                                        scale=SCALE,
                                    )
                                    if rel >= 0:
                                        nc.vector.tensor_mul(
                                            pexp[:, o : o + 128],
                                            pexp[:, o : o + 128],
                                            tri_sb,
                                        )

                                    def av_unit(
                                        h=h, jb=jb, o=o, jd=jd, jmax=jmax,
                                        pO=pO, pexp=pexp,
                                    ):
                                        # seg1 first (never overlaps the
                                        # masked triangle); exact causal trim
                                        for s in (1, 0):
                                            hi = 512 * (s + 1)
                                            lo = max(o, 512 * s)
                                            if lo >= hi:
                                                continue
                                            vsl = slice(
                                                jb * HPC * VW + VW * h,
                                                jb * HPC * VW + VW * (h + 1),
                                            )
                                            nc.tensor.matmul(
                                                pO[:, lo:hi],
                                                lhsT=v_all[:, vsl],
                                                rhs=pexp[:, lo:hi],
                                                start=(jb == 0),
                                                stop=(
                                                    jb
                                                    == (jd + 3 if s == 0 else jmax - 1)
                                                ),
                                                skip_group_check=True,
                                            )

                                    pend.append(av_unit)
                                    drain(DELAY)
                                    yield (h, cp, jb)

                                # Normalize: split the O^T psum into two
                                # partition-0-aligned SBUF tiles (O rows on
                                # ACT, sum rows on DVE — reciprocal_approx_
                                # fast's custom ucode can't take a partition
                                # offset), one approx-reciprocal over the
                                # whole sums tile, then the broadcast
                                # multiplies in 256-col chunks spread through
                                # the pend queue.
                                pOcO = att_pool.tile(
                                    [64, IC], f32, name="pOcO", tag="pOcO", bufs=2
                                )
                                pOcS = att_pool.tile(
                                    [64, IC], f32, name="pOcS", tag="pOcS", bufs=2
                                )
                                rec = att_pool.tile(
                                    [64, IC], f32, name="rec", tag="rec", bufs=2
                                )

                                def copy_unit(pO=pO, pOcO=pOcO, pOcS=pOcS, rec=rec):
                                    nc.scalar.copy(out=pOcO, in_=pO[0:64, :])
                                    nc.vector.tensor_copy(
                                        out=pOcS, in_=pO[64:128, :]
                                    )
                                    nc.vector.reciprocal_approx_fast(
                                        out=rec, in_=pOcS
                                    )

                                pend.append(copy_unit)
                                for c4 in range(4):
                                    def norm_chunk(
                                        g=g, row=row, cp=cp, pOcO=pOcO,
                                        rec=rec, c4=c4,
                                    ):
                                        cs = slice(256 * c4, 256 * (c4 + 1))
                                        nc.vector.tensor_mul(
                                            OT[g][
                                                row : row + 64,
                                                IC * cp + 256 * c4 : IC * cp
                                                + 256 * (c4 + 1),
                                            ],
                                            pOcO[:, cs],
                                            rec[:, cs],
                                        )

                                    pend.append(norm_chunk)

                    att = attention_stream()
                    vfill = vproj_stream(range(8, 16))
                    g1fill = qkproj_stream(1, copy_engines=("vector",))
                    out0fill = outproj_stream(range(8), ("vector",))
                    fillers = [g0fill, vfill]
                    g1_done = False
                    steps = 0
                    for h, cp, jb in att:
                        steps += 1
                        pulled = False
                        while fillers and not pulled:
                            if next(fillers[0], "end") == "end":
                                fillers.pop(0)
                            else:
                                pulled = True
                        if not pulled and not g1_done and steps % 3 == 0:
                            g1_done = next(g1fill, "end") == "end"
                        if h == 2 and not g1_done:
                            # g1 attention needs qT[1]/kT[1] complete
                            for _ in g1fill:
                                pass
                            g1_done = True
                        # nb 0..7 of the output projection read cp0 columns of
                        # OT; head 3's cp0 normalize leaves the pend queue at
                        # jb==DELAY of cp1, so only start pulling after that
                        if h == 3 and cp == 1 and jb > DELAY:
                            next(out0fill, None)
                            next(out0fill, None)
                    # drain the tail interleaved with the final output
                    # blocks: norm chunk c4 of (h3, cp1) unlocks output
                    # columns 256*c4..256*(c4+1), i.e. nb 8+2*c4, 9+2*c4
                    outfin = outproj_stream(range(8, 16), ("scalar", "vector"))
                    while pend:
                        pend.pop(0)()
                        if len(pend) < 4:
                            for _ in range(4):
                                next(outfin, None)
                    for _ in out0fill:
                        pass
                    for _ in outfin:
                        pass

    nc.compile()
    return nc


def kernel(x, mask, Wq, Wkv, Wout, b_out):
    global _last_results
    import ml_dtypes
    from concourse.bass_utils import run_bass_kernel_spmd

    bf = ml_dtypes.bfloat16
    x = np.asarray(x, dtype=np.float32).astype(bf)
    Wq = np.asarray(Wq, dtype=np.float32).astype(bf)
    Wkv = np.asarray(Wkv, dtype=np.float32).astype(bf)
    Wout = np.asarray(Wout, dtype=np.float32).astype(bf)
    b_out = np.asarray(b_out, dtype=np.float32)

    if "nc" not in _cached:
        _cached["nc"] = _build_program()
    nc = _cached["nc"]

    jj, ii = np.mgrid[0:128, 0:128]
    tri = (jj <= ii).astype(bf)

    in_maps = []
    for c in range(NCORES):
        b = c // 4
        h0 = HPC * (c % 4)
        in_maps.append(
            {
                "xb": np.ascontiguousarray(x[b]),
                "wq": np.ascontiguousarray(Wq[:, DH * h0 : DH * (h0 + HPC)]),
                "wk": np.ascontiguousarray(Wkv[:, DH * h0 : DH * (h0 + HPC)]),
                "wv": np.ascontiguousarray(
                    Wkv[:, D + DH * h0 : D + DH * (h0 + HPC)]
                ),
                "wo": np.ascontiguousarray(Wout[DH * h0 : DH * (h0 + HPC), :]),
                "tri": tri,
            }
        )

    res = run_bass_kernel_spmd(
        nc,
        in_maps,
        core_ids=list(range(NCORES)),
        trace=bool(int(os.environ.get("KERNEL_TRACE", "0"))),
    )
    _last_results = res
    parts = [r["outp"] for r in res.results]
    out = np.empty((B, N, D), dtype=np.float32)
    for b in range(B):
        acc = np.asarray(parts[4 * b], dtype=np.float32)
        for c in range(4 * b + 1, 4 * b + 4):
            acc += np.asarray(parts[c], dtype=np.float32)
        out[b] = acc + b_out[None, :]
    return out


# revision 22
# speedup vs baseline: 1.4132x; 1.0208x over previous
"""Causal multi-head attention kernel for 8 trn2 NeuronCores.

Problem: x[2,2048,1024], 16 heads of dim 64, causal softmax(q k^T / sqrt(1024)) v,
then output projection. Sharding: data-parallel over batch (4 cores per batch),
tensor-parallel over heads (4 heads per core). Each core produces a partial
output (its heads' contribution through Wout); the host sums the 4 partials per
batch and adds b_out.

Datapath is bf16 (inputs cast on host) with fp32 PSUM accumulation; the
rel-err budget is 2e-2 and bf16 keeps the end-to-end error ~5e-3. bf16 buys:
full PE rate at any free size (no >=256 fp32r padding), DMA-engine XBAR
transpose of x (the whole PE transpose phase is gone), half the DMA bytes,
and 2x DVE modes on the mask multiplies.

Per-core device program (SPMD, per-core data arrives via input tensors):
  1. xT [d on partitions, n free] arrives directly via dma_start_transpose
     (16 half-slab XBAR transposes), overlapped with weight DMAs.
  2. Projections: qT/kT [dh on partitions, n free] (2-head groups of 128
     partitions), v natural [n on partitions] augmented with a ones column so
     the attention matmul also produces softmax row-sums.
  3. Per head, per 1024-wide i-chunk: S^T[j-block, i] = kT_j^T . qT_i on PE,
     exp((1/32) S) on ACT straight out of PSUM (logits are bounded, no
     max-subtraction needed) writing bf16, triangular mask multiply only on
     the diagonal 128x128 block, then O^T[dh+1, i] += V_aug^T . P^T
     accumulated over j-blocks in PSUM. Block-causality skips all j>i blocks,
     and segments are trimmed exactly to the causal offset.
  4. Normalize by the row-sums (reciprocal_approx_fast + broadcast multiply),
     output projection with two 2-head pairs packed to a full K=128
     contraction.
"""

import os

import numpy as np

B, N, D, H = 2, 2048, 1024, 16
DH = D // H  # 64
SCALE = float(D) ** -0.5
NCORES = 8
HPC = 4  # heads per core
IC = 1024  # i-chunk width in attention phase
NB = N // 128  # 16 j/n blocks
KT = D // 128  # 8 contraction tiles
# v columns per head: 64 data cols + 64 ones cols. The ones columns make the
# attention matmul write the softmax row-sum replicated on PSUM partitions
# 64..127, so normalization is a plain elementwise reciprocal+multiply
# (partitions 0..63 / partitions 64..127) with no partition-broadcast needed.
VW = 2 * DH  # 128

_cached = {}
_last_results = None


def _build_program():
    import concourse.bacc as bacc
    import concourse.mybir as mybir
    import concourse.tile as tile

    f32 = mybir.dt.float32
    bf16 = mybir.dt.bfloat16
    EXP = mybir.ActivationFunctionType.Exp

    nc = bacc.Bacc()

    xb = nc.dram_tensor("xb", [N, D], bf16, kind="ExternalInput")
    wq = nc.dram_tensor("wq", [D, HPC * DH], bf16, kind="ExternalInput")
    wk = nc.dram_tensor("wk", [D, HPC * DH], bf16, kind="ExternalInput")
    wv = nc.dram_tensor("wv", [D, HPC * DH], bf16, kind="ExternalInput")
    wo = nc.dram_tensor("wo", [HPC * DH, D], bf16, kind="ExternalInput")
    tri = nc.dram_tensor("tri", [128, 128], bf16, kind="ExternalInput")
    outp = nc.dram_tensor("outp", [N, D], bf16, kind="ExternalOutput")

    with tile.TileContext(nc) as tc:
        with (
            tc.tile_pool(name="const", bufs=1) as const_pool,
            tc.tile_pool(name="wts", bufs=1) as wts_pool,
            tc.tile_pool(name="big", bufs=1) as big_pool,
        ):
            # Input DMA is the kernel lead-in and every ns of it delays the
            # PE's first matmul, so it is split across both HWDGE queues
            # (SP="sync" and ACT="scalar") in consumption order: wv first
            # (the V projection runs first), then the x^T XBAR-transpose
            # half-slabs interleaved with wq/wk, and the attention-only
            # constants (tri, wo) last.
            tri_sb = const_pool.tile([128, 128], bf16, name="tri_sb", tag="tri_sb")
            wo_all = const_pool.tile([128, 2 * D], bf16, name="wo_all", tag="wo_all")
            wo_sb = [wo_all[:, D * g : D * (g + 1)] for g in range(2)]
            # each [D, 256] weight loads as ONE DMA into [128, 8*256] (the 8
            # 128-row D-tiles side by side) — 1 queue slot instead of 8
            wq_sb, wk_sb, wv_sb = [], [], []
            w_alls = {}
            for nm, lst in (("wq", wq_sb), ("wk", wk_sb), ("wv", wv_sb)):
                t = wts_pool.tile(
                    [128, KT * HPC * DH], bf16, name=f"{nm}_all", tag=f"{nm}_all"
                )
                w_alls[nm] = t
                for r in range(KT):
                    lst.append(t[:, HPC * DH * r : HPC * DH * (r + 1)])
            xT = []
            for r in range(KT):
                t = big_pool.tile([128, N], bf16, name=f"xT{r}", tag=f"xT{r}")
                xT.append(t)

            # Everything goes on ONE DMA queue: concurrent traffic on the
            # second HWDGE queue poisons the DMA cadence (6us per transfer
            # instead of 1.3us) and concurrent XBAR transposes on two queues
            # corrupt each other. Order is consumption order, so the PE's
            # first matmul only waits for wv + the half0 transposes.
            nc.sync.dma_start(
                out=w_alls["wv"].rearrange("p (r c) -> p r c", r=KT),
                in_=wv[:, :].rearrange("(r p) c -> p r c", r=KT),
            )
            for r in range(KT):
                nc.sync.dma_start_transpose(
                    out=xT[r][:, 0:1024], in_=xb[0:1024, 128 * r : 128 * (r + 1)]
                )
            nc.sync.dma_start(
                out=w_alls["wq"].rearrange("p (r c) -> p r c", r=KT),
                in_=wq[:, :].rearrange("(r p) c -> p r c", r=KT),
            )
            nc.sync.dma_start(
                out=w_alls["wk"].rearrange("p (r c) -> p r c", r=KT),
                in_=wk[:, :].rearrange("(r p) c -> p r c", r=KT),
            )
            nc.sync.dma_start(out=tri_sb, in_=tri[:, :])
            for r in range(KT):
                nc.sync.dma_start_transpose(
                    out=xT[r][:, 1024:2048],
                    in_=xb[1024:2048, 128 * r : 128 * (r + 1)],
                )
            nc.sync.dma_start(
                out=wo_all.rearrange("p (g c) -> p g c", g=2),
                in_=wo[:, :].rearrange("(g p) c -> p g c", g=2),
            )

            with (
                tc.tile_pool(name="pj", bufs=2, space="PSUM") as pj_pool,
                tc.tile_pool(name="osb", bufs=3) as osb_pool,
            ):
                qT, kT_ = [], []
                for g in range(2):
                    tq = big_pool.tile([128, N], bf16, name=f"qT{g}", tag=f"qT{g}")
                    tk = big_pool.tile([128, N], bf16, name=f"kT{g}", tag=f"kT{g}")
                    qT.append(tq)
                    kT_.append(tk)
                v_all = big_pool.tile(
                    [128, NB * HPC * VW], bf16, name="v_all", tag="v_all"
                )
                # ones columns for the row-sum trick: fill the whole tile with
                # 1.0; the projection copies below overwrite the data columns
                nc.vector.memset(v_all, 1.0)


                def _copy(eng, out, in_):
                    if eng == "scalar":
                        nc.scalar.copy(out=out, in_=in_)
                    else:
                        getattr(nc, eng).tensor_copy(out=out, in_=in_)

                def vproj_stream(nbs):
                    for nb in nbs:
                        pv = pj_pool.tile([128, HPC * DH], f32, name="pv", tag="pj")
                        for r in range(KT):
                            nc.tensor.matmul(
                                pv,
                                lhsT=xT[r][:, 128 * nb : 128 * (nb + 1)],
                                rhs=wv_sb[r],
                                start=(r == 0),
                                stop=(r == KT - 1),
                            )
                        base = nb * HPC * VW
                        for h in range(HPC):
                            _copy(
                                ("vector", "scalar", "scalar", "vector")[h],
                                v_all[:, base + VW * h : base + VW * h + DH],
                                pv[:, DH * h : DH * (h + 1)],
                            )
                        yield

                def qkproj_stream(g, copy_engines=("any",)):
                    ci = 0
                    for s4 in range(4):
                        sl = slice(512 * s4, 512 * (s4 + 1))
                        for w_sb, dst in ((wq_sb, qT[g]), (wk_sb, kT_[g])):
                            pq = pj_pool.tile([128, 512], f32, name="pq", tag="pj")
                            for r in range(KT):
                                nc.tensor.matmul(
                                    pq,
                                    lhsT=w_sb[r][:, 128 * g : 128 * (g + 1)],
                                    rhs=xT[r][:, sl],
                                    start=(r == 0),
                                    stop=(r == KT - 1),
                                )
                            _copy(copy_engines[ci % len(copy_engines)], dst[:, sl], pq)
                            ci += 1
                            yield

                # ---------------- phase B: projections ----------------
                # emission order tracks the DMA half-slab arrival order; only
                # what attention head 0 cp0 needs (V nb0..7 + all of qk g0)
                # runs here — the rest of V and all of g1 fill PE slack
                # inside the attention loop.
                for _ in vproj_stream(range(8)):
                    pass
                g0fill = qkproj_stream(0, copy_engines=("vector", "scalar"))
                for _ in range(4):
                    next(g0fill)

                # ---------------- phase C: attention with interleaved
                # ---------------- g1 projections and output projection ------
                OT = []
                for g in range(2):
                    t = big_pool.tile([128, N], bf16, name=f"OT{g}", tag=f"OT{g}")
                    OT.append(t)

                def outproj_stream(nbs, copy_engines):
                    ci = 0
                    for nb in nbs:
                        nsl = slice(128 * nb, 128 * (nb + 1))
                        ob = osb_pool.tile([128, D], bf16, name="ob", tag="osb")
                        for s in range(2):
                            po = pj_pool.tile([128, 512], f32, name="po", tag="pj")
                            for g in range(2):
                                nc.tensor.matmul(
                                    po,
                                    lhsT=OT[g][:, nsl],
                                    rhs=wo_sb[g][:, 512 * s : 512 * (s + 1)],
                                    start=(g == 0),
                                    stop=(g == 1),
                                )
                            _copy(
                                copy_engines[ci % len(copy_engines)],
                                ob[:, 512 * s : 512 * (s + 1)],
                                po,
                            )
                            ci += 1
                            if s == 1:
                                nc.sync.dma_start(out=outp[nsl, :], in_=ob)
                            yield

                with (
                    tc.tile_pool(name="pS", bufs=2, space="PSUM") as pS_pool,
                    tc.tile_pool(name="pO", bufs=1, space="PSUM") as pO_pool,
                    tc.tile_pool(name="att", bufs=4) as att_pool,
                ):
                    # A@V emission lags the QK/exp emission by DELAY jb-steps
                    # so the in-order PE never stalls on the ACT exp; the
                    # PE-dense projection streams above are pulled in between
                    # attention steps to fill the remaining PE idle time.
                    DELAY = 3
                    pend = []

                    def drain(n):
                        while len(pend) > n:
                            pend.pop(0)()

                    # group order: cp0 of a head pair first, then their cp1
                    # (cp1 needs the second half-slab DMAs), then the next
                    # pair; out-proj for i<1024 can start once (3,0) is done
                    ORDER = [
                        (0, 0), (1, 0), (0, 1), (1, 1),
                        (2, 0), (3, 0), (2, 1), (3, 1),
                    ]

                    def attention_stream():
                        for h, cp in ORDER:
                            g, row = h // 2, 64 * (h % 2)
                            if True:
                                jd, jmax = 8 * cp, 8 * (cp + 1)
                                pO = pO_pool.tile(
                                    [128, IC], f32, name=f"pO{h}", tag="pO"
                                )
                                for jb in range(jmax):
                                    rel = jb - jd
                                    o = 128 * rel if rel > 0 else 0
                                    jsl = slice(128 * jb, 128 * (jb + 1))
                                    pS = pS_pool.tile(
                                        [128, IC], f32, name="pS", tag="pS"
                                    )
                                    pexp = att_pool.tile(
                                        [128, IC], bf16, name="pexp", tag="pexp",
                                        bufs=5,
                                    )
                                    # S^T = kT_j^T . qT_i in 512-wide segments
                                    # trimmed exactly to the causal offset
                                    for s in range(2):
                                        a = max(o, 512 * s)
                                        if a >= 512 * (s + 1):
                                            continue
                                        nc.tensor.matmul(
                                            pS[:, a : 512 * (s + 1)],
                                            lhsT=kT_[g][row : row + 64, jsl],
                                            rhs=qT[g][
                                                row : row + 64,
                                                IC * cp + a : IC * cp + 512 * (s + 1),
                                            ],
                                            start=True,
                                            stop=True,
                                        )
                                    nc.scalar.activation(
                                        out=pexp[:, o:IC],
                                        in_=pS[:, o:IC],
                                        func=EXP,
                                        scale=SCALE,
                                    )
                                    if rel >= 0:
                                        nc.vector.tensor_mul(
                                            pexp[:, o : o + 128],
                                            pexp[:, o : o + 128],
                                            tri_sb,
                                        )

                                    def av_unit(
                                        h=h, jb=jb, o=o, jd=jd, jmax=jmax,
                                        pO=pO, pexp=pexp,
                                    ):
                                        # seg1 first (never overlaps the
                                        # masked triangle); exact causal trim
                                        for s in (1, 0):
                                            hi = 512 * (s + 1)
                                            lo = max(o, 512 * s)
                                            if lo >= hi:
                                                continue
                                            vsl = slice(
                                                jb * HPC * VW + VW * h,
                                                jb * HPC * VW + VW * (h + 1),
                                            )
                                            nc.tensor.matmul(
                                                pO[:, lo:hi],
                                                lhsT=v_all[:, vsl],
                                                rhs=pexp[:, lo:hi],
                                                start=(jb == 0),
                                                stop=(
                                                    jb
                                                    == (jd + 3 if s == 0 else jmax - 1)
                                                ),
                                                skip_group_check=True,
                                            )

                                    pend.append(av_unit)
                                    drain(DELAY)
                                    yield (h, cp, jb)

                                # Normalize: split the O^T psum into two
                                # partition-0-aligned SBUF tiles (O rows on
                                # ACT, sum rows on DVE — reciprocal_approx_
                                # fast's custom ucode can't take a partition
                                # offset), one approx-reciprocal over the
                                # whole sums tile, then the broadcast
                                # multiplies in 256-col chunks spread through
                                # the pend queue.
                                pOcO = att_pool.tile(
                                    [64, IC], f32, name="pOcO", tag="pOcO", bufs=2
                                )
                                pOcS = att_pool.tile(
                                    [64, IC], f32, name="pOcS", tag="pOcS", bufs=2
                                )
                                rec = att_pool.tile(
                                    [64, IC], f32, name="rec", tag="rec", bufs=2
                                )

                                def copy_unit(pO=pO, pOcO=pOcO, pOcS=pOcS, rec=rec):
                                    nc.scalar.copy(out=pOcO, in_=pO[0:64, :])
                                    nc.vector.tensor_copy(
                                        out=pOcS, in_=pO[64:128, :]
                                    )
                                    nc.vector.reciprocal_approx_fast(
                                        out=rec, in_=pOcS
                                    )

                                pend.append(copy_unit)
                                for c4 in range(4):
                                    def norm_chunk(
                                        g=g, row=row, cp=cp, pOcO=pOcO,
                                        rec=rec, c4=c4,
                                    ):
                                        cs = slice(256 * c4, 256 * (c4 + 1))
                                        nc.vector.tensor_mul(
                                            OT[g][
                                                row : row + 64,
                                                IC * cp + 256 * c4 : IC * cp
                                                + 256 * (c4 + 1),
                                            ],
                                            pOcO[:, cs],
                                            rec[:, cs],
                                        )

                                    pend.append(norm_chunk)

                    att = attention_stream()
                    vfill = vproj_stream(range(8, 16))
                    g1fill = qkproj_stream(1, copy_engines=("vector",))
                    out0fill = outproj_stream(range(8), ("vector",))
                    fillers = [g0fill, vfill]
                    g1_done = False
                    steps = 0
                    for h, cp, jb in att:
                        steps += 1
                        pulled = False
                        while fillers and not pulled:
                            if next(fillers[0], "end") == "end":
                                fillers.pop(0)
                            else:
                                pulled = True
                        if not pulled and not g1_done and steps % 3 == 0:
                            g1_done = next(g1fill, "end") == "end"
                        if h == 2 and not g1_done:
                            # g1 attention needs qT[1]/kT[1] complete
                            for _ in g1fill:
                                pass
                            g1_done = True
                        # nb 0..7 of the output projection read cp0 columns
                        # of OT, complete once (3,0)'s normalize drains out
                        # of the pend queue early in (2,1)
                        if (h == 2 and cp == 1 and jb > DELAY + 2) or (
                            h == 3 and cp == 1
                        ):
                            next(out0fill, None)
                            next(out0fill, None)
                    # drain the tail interleaved with the final output
                    # blocks: norm chunk c4 of (h3, cp1) unlocks output
                    # columns 256*c4..256*(c4+1), i.e. nb 8+2*c4, 9+2*c4
                    outfin = outproj_stream(range(8, 16), ("scalar", "vector"))
                    while pend:
                        pend.pop(0)()
                        if len(pend) < 4:
                            for _ in range(4):
                                next(outfin, None)
                    for _ in out0fill:
                        pass
                    for _ in outfin:
                        pass

    nc.compile()
    return nc


def kernel(x, mask, Wq, Wkv, Wout, b_out):
    global _last_results
    import ml_dtypes
    from concourse.bass_utils import run_bass_kernel_spmd

    bf = ml_dtypes.bfloat16
    x = np.asarray(x, dtype=np.float32).astype(bf)
    Wq = np.asarray(Wq, dtype=np.float32).astype(bf)
    Wkv = np.asarray(Wkv, dtype=np.float32).astype(bf)
    Wout = np.asarray(Wout, dtype=np.float32).astype(bf)
    b_out = np.asarray(b_out, dtype=np.float32)

    if "nc" not in _cached:
        _cached["nc"] = _build_program()
    nc = _cached["nc"]

    jj, ii = np.mgrid[0:128, 0:128]
    tri = (jj <= ii).astype(bf)

    in_maps = []
    for c in range(NCORES):
        b = c // 4
        h0 = HPC * (c % 4)
        in_maps.append(
            {
                "xb": np.ascontiguousarray(x[b]),
                "wq": np.ascontiguousarray(Wq[:, DH * h0 : DH * (h0 + HPC)]),
                "wk": np.ascontiguousarray(Wkv[:, DH * h0 : DH * (h0 + HPC)]),
                "wv": np.ascontiguousarray(
                    Wkv[:, D + DH * h0 : D + DH * (h0 + HPC)]
                ),
                "wo": np.ascontiguousarray(Wout[DH * h0 : DH * (h0 + HPC), :]),
                "tri": tri,
            }
        )

    res = run_bass_kernel_spmd(
        nc,
        in_maps,
        core_ids=list(range(NCORES)),
        trace=bool(int(os.environ.get("KERNEL_TRACE", "0"))),
    )
    _last_results = res
    parts = [r["outp"] for r in res.results]
    out = np.empty((B, N, D), dtype=np.float32)
    for b in range(B):
        acc = np.asarray(parts[4 * b], dtype=np.float32)
        for c in range(4 * b + 1, 4 * b + 4):
            acc += np.asarray(parts[c], dtype=np.float32)
        out[b] = acc + b_out[None, :]
    return out
